# revision 11
# baseline (speedup 1.0000x reference)
"""Trainium2 Bass kernel for nn_Attention (B=2, L=2048, DIM=1024, H=16, D=64).

Sharding: 8 cores, each handles one (b, 4-head-group) pair — data parallel
on B (cores 0-3 -> b=0, cores 4-7 -> b=1), tensor parallel on heads
(4 heads per core). The output projection is computed per-core over the
core's 4 heads; the host sums the 4 partials per batch and adds the bias.

All matmuls run in float32r (single-pass fp32 PE mode, ~bf16 speed at
N>=512, ~1.6e-4 matmul rel err). DRAM inputs are declared float32r
directly (bit-identical f32 storage; the PE truncates instead of rounds
the mantissa, a negligible 2^-19 difference), so every load is a plain
DMA with no staging or cast step.

Device-side layout (per core, contraction dim on SBUF partitions):
  xT   [DIM, L]     x[b]^T
  wqk  [DIM, 512]   [Wq_scaled | Wk]^T for the core's 4 heads (Wq pre-scaled
                    by qk_scale * s * log(L) so exp() needs no extra scale)
  wv   [DIM, 256]   Wv^T for the 4 heads
  wp   [256, DIM]   proj_w[:, head_slice]^T
  y    [L, DIM]     per-core partial output (pre-bias)

Schedule (PE-bubble-free to avoid HAM duty-cycle re-throttle: any PE idle
window >3.4us drops the PE clock from 2.4 to 1.2 GHz for the next ~3.4us+;
the warmup matmuls are sized so the clock is hot when wqk lands):
 - QKV phase consumes x^T chunk-by-chunk in DMA arrival order: per l-chunk
   {q ft0, k ft2, V 4 l-tiles}, then q/k for heads 2-3 (ft1/ft3) at the
   tail of the (DMA-bound) phase. Inputs ride two concurrent DMA queues
   with one dma_start per tensor chunk (per-dma dispatch costs ~1us on
   the SWDGE queue; small DMAs starve the matmul stream).
 - Attention per head pair: S^T tiles for both heads land in one
   [128,1024] PSUM pair tile (the two K=64 matmuls dual-issue on PE row
   groups h0/h64, so S costs half its naive time); one ACT exp converts
   the pair; A*V via matmul(lhsT=[V | ones], rhs=P^T) also yields softmax
   denominators in row D of the PSUM accumulator. The attention steady
   state is ACT-exp-bound (~1.13us per m-tile, ~99% ACT occupancy).
 - Normalize with no DRAM roundtrip and no PE involvement: the [O; denom]
   block leaves PSUM via one DVE copy (releasing the po bank so the next
   head pair's A*V never stalls), DVE reciprocal_approx_fast inverts the
   denominator row (input staged to partition 0 — the custom-DVE op
   misreads partition-offset APs on HW), gpsimd partition_broadcast fans
   it out, one DVE multiply writes oT. No DMA queue or PE in the chain,
   so chunk-boundary PE stalls stay well under the HAM window.
 - The projection is software-pipelined one l-chunk behind, interleaved
   into the next chunk's matmul stream; oT lives in per-chunk pool tiles
   so chunk i's projection has no false dependency on chunk i+1's
   normalize. The very last normalize chain skips the PSUM-release copy
   and multiplies straight from PSUM to shorten the tail.
"""

import math
import sys

sys.path.insert(0, "/opt/trn_rl_repo")

import numpy as np

import concourse.bass as bass
import concourse.tile as tile
from concourse import bacc, bass_utils, mybir

B, L, DIM, H, D = 2, 2048, 1024, 16, 64
N_CORES = 8
HL = 4  # heads per core
F = HL * D  # 256: per-core head feature width
LC, LT, CT = 512, 128, 128  # l-chunk, l/m-tile, contraction tile
N_LC, N_LT, N_CT = L // LC, L // LT, DIM // CT

DT = mybir.dt.float32r
BF16 = mybir.dt.bfloat16
F32 = mybir.dt.float32

_build_cache = {}


def _build(with_mask: bool):
    if with_mask in _build_cache:
        return _build_cache[with_mask]

    nc = bacc.Bacc("TRN2", target_bir_lowering=False, debug=False, num_devices=N_CORES)
    # inputs declared float32r: bit-identical f32 storage (np maps both to
    # float32), so plain DMAs load them with no cast step. The PE's f32r
    # mode truncates instead of rounds the mantissa — a 2^-19 relative
    # difference, far below the matmul's own ~1.6e-4 error.
    xT = nc.dram_tensor("xT", [DIM, L], DT, kind="ExternalInput").ap()
    wqk = nc.dram_tensor("wqk", [DIM, 2 * F], DT, kind="ExternalInput").ap()
    wv = nc.dram_tensor("wv", [DIM, F], DT, kind="ExternalInput").ap()
    wp = nc.dram_tensor("wp", [F, DIM], DT, kind="ExternalInput").ap()
    if with_mask:
        maskT = nc.dram_tensor("maskT", [HL, L, L], F32, kind="ExternalInput").ap()
    y = nc.dram_tensor("y", [L, DIM], F32, kind="ExternalOutput").ap()

    Exp = mybir.ActivationFunctionType.Exp

    with tile.TileContext(nc) as tc:
        with (
            tc.tile_pool(name="consts", bufs=1) as consts,
            tc.tile_pool(name="work", bufs=3) as work,
            tc.tile_pool(name="otp", bufs=2) as otp,
            tc.tile_pool(name="ps_mm", bufs=3, space="PSUM") as ps_mm,
            tc.tile_pool(name="ps_acc", bufs=2, space="PSUM") as ps_acc,
        ):
            # ---- PE warmup: dummy matmuls during input DMA so the HAM
            # clock-gate reaches 2.4 GHz before the real work starts ----
            warm = consts.tile([128, 512], mybir.dt.bfloat16)
            nc.vector.memset(warm, 0.0)
            ps_w = ps_acc.tile([128, 512], F32, name="ps_w", tag="acc")
            n_warm = 46  # sized to end as wqk lands (~21us): the clock is
            # warm when real work starts, with no wasted PE time after
            for i in range(n_warm):
                nc.tensor.matmul(
                    ps_w, lhsT=warm[:, 0:128], rhs=warm,
                    start=(i == 0), stop=(i == n_warm - 1),
                )

            # ---- load inputs. All tensors are f32r in DRAM (bit-identical
            # f32), so every load is a plain DMA — no staging, no DVE cast.
            # Split across the two queues so they pull concurrently; one
            # dma_start per tensor chunk (per-dma dispatch costs ~1us on
            # the SWDGE queue). Order matters: HBM bandwidth is the
            # limiter, so urgent first. ----
            xT_sb = consts.tile([128, N_CT, L], DT)
            wv_sb = consts.tile([128, N_CT, F], DT)
            wp_sb = consts.tile([128, 2, DIM], DT)
            wqk_sb = consts.tile([128, N_CT, 2 * F], DT)
            # gpsimd: wqk first (gates all q/k groups), wv (needed by the
            # first V tiles), then x^T lc2-3, wp last (needed mid-attention)
            for h in range(2):
                hc = N_CT // 2
                src2 = bass.AP(
                    tensor=wqk.tensor,
                    offset=h * hc * 128 * 2 * F,
                    ap=[[2 * F, 128], [128 * 2 * F, hc], [1, 2 * F]],
                )
                nc.gpsimd.dma_start(out=wqk_sb[:, h * hc : (h + 1) * hc, :], in_=src2)
            srcv = bass.AP(
                tensor=wv.tensor,
                offset=0,
                ap=[[F, 128], [128 * F, N_CT], [1, F]],
            )
            nc.gpsimd.dma_start(out=wv_sb, in_=srcv)
            # sync: x^T lc0 and lc1
            for lc in range(2):
                srcx = bass.AP(
                    tensor=xT.tensor,
                    offset=lc * LC,
                    ap=[[L, 128], [128 * L, N_CT], [1, LC]],
                )
                nc.sync.dma_start(
                    out=xT_sb[:, :, lc * LC : (lc + 1) * LC], in_=srcx
                )
            for lc in range(2, N_LC):
                srcx = bass.AP(
                    tensor=xT.tensor,
                    offset=lc * LC,
                    ap=[[L, 128], [128 * L, N_CT], [1, LC]],
                )
                nc.gpsimd.dma_start(out=xT_sb[:, :, lc * LC : (lc + 1) * LC], in_=srcx)
            nc.gpsimd.dma_start(out=wp_sb, in_=wp[0:256, :].rearrange("(t p) o -> p t o", t=2))

            # ---- stage A: Q^T/K^T [f, l] (f = [q 4 heads | k 4 heads] * 64)
            # and V, interleaved per l-chunk in DMA arrival order ----
            qkT_sb = consts.tile([128, 4, L], DT)
            v_sb = consts.tile([128, N_LT, HL, D + 1], DT)
            ones_f32 = consts.tile([128, 64], F32)
            nc.vector.memset(ones_f32, 1.0)
            nc.vector.tensor_copy(
                v_sb[:, :, :, D : D + 1],
                ones_f32.rearrange("p (a b c) -> p a b c", a=N_LT, b=HL),
            )

            def qk_group(ft, lc):
                ps = ps_mm.tile([128, LC], F32, name="ps_qk", tag="mm")
                for c in range(N_CT):
                    nc.tensor.matmul(
                        ps,
                        lhsT=wqk_sb[:, c, ft * 128 : (ft + 1) * 128],
                        rhs=xT_sb[:, c, lc * LC : (lc + 1) * LC],
                        start=(c == 0),
                        stop=(c == N_CT - 1),
                    )
                nc.vector.tensor_copy(qkT_sb[:, ft, lc * LC : (lc + 1) * LC], ps)

            def v_tile(lt):
                ps = ps_mm.tile([128, F], F32, name="ps_v", tag="mm")
                for c in range(N_CT):
                    nc.tensor.matmul(
                        ps,
                        lhsT=xT_sb[:, c, lt * 128 : (lt + 1) * 128],
                        rhs=wv_sb[:, c, :],
                        start=(c == 0),
                        stop=(c == N_CT - 1),
                    )
                nc.vector.tensor_copy(
                    v_sb[:, lt, :, 0:D], ps.rearrange("p (h d) -> p h d", h=HL)
                )

            for lc in range(N_LC):
                qk_group(0, lc)
                qk_group(2, lc)
                for lt in range(lc * (LC // 128), (lc + 1) * (LC // 128)):
                    v_tile(lt)

            # q/k for heads 2-3 at the tail of the (DMA-bound) QKV phase:
            # they need no new data, and keeping them out of the attention
            # phase keeps the per-m-tile PE load at or below the ACT exp
            # rate there (the attention steady state is ACT-paced)
            for ft in (1, 3):
                for lc in range(N_LC):
                    qk_group(ft, lc)

            # ---- stage B + C fused: attention, with the projection software-
            # pipelined one l-chunk behind so the PE never stalls on the
            # normalize chain (stalls re-throttle the PE clock via HAM) ----
            def project_group(oT, lt, direct=False):
                # one [128, 1024] output row-tile per group; each matmul
                # output stays within a single 2KB PSUM bank (N=512 max)
                ltl = lt % (LC // 128)
                ps = ps_mm.tile([128, DIM], F32, name="ps_y", tag="mm")
                for oc in range(2):
                    osl = slice(oc * 512, (oc + 1) * 512)
                    for ft in range(2):
                        nc.tensor.matmul(
                            ps[:, osl],
                            lhsT=oT[:, ft, ltl * 128 : (ltl + 1) * 128],
                            rhs=wp_sb[:, ft, osl],
                            start=(ft == 0),
                            stop=(ft == 1),
                        )
                yb = work.tile([128, DIM], F32, name="yb", tag="yb", bufs=2)
                if direct:
                    # final groups: the ACT engine is idle by now, so stage
                    # through it instead of queueing behind the DVE copies
                    nc.scalar.copy(yb, ps)
                else:
                    nc.vector.tensor_copy(yb, ps)
                nc.sync.dma_start(out=y[lt * 128 : (lt + 1) * 128, :], in_=yb)

            # (oT, lt, oc) projection groups for the previous l-chunk, emitted
            # interleaved into the current chunk's matmul stream
            pending_proj = []
            for lc in range(N_LC):
                lsl = slice(lc * LC, (lc + 1) * LC)
                oT = otp.tile([128, 2, LC], DT, name="oT", tag="ot")
                for hp in range(2):  # head pairs (2*hp, 2*hp+1)
                    po = [
                        ps_acc.tile([128, LC], F32, name="po", tag="acc")
                        for _ in range(2)
                    ]
                    ps_s_q = []

                    def s_pair(mt):
                        msl = slice(mt * 128, (mt + 1) * 128)
                        ps_s = ps_mm.tile([128, 2 * LC], F32, name="ps_s", tag="mm")
                        for hh in range(2):
                            off = 64 * hh
                            nc.tensor.matmul(
                                ps_s[:, hh * LC : (hh + 1) * LC],
                                lhsT=qkT_sb[off : off + 64, 2 + hp, msl],
                                rhs=qkT_sb[off : off + 64, hp, lsl],
                                start=True,
                                stop=True,
                            )
                        if with_mask:
                            for hh in range(2):
                                h = 2 * hp + hh
                                mk = work.tile([128, LC], F32, name="mk", tag="mk", bufs=4)
                                nc.sync.dma_start(out=mk, in_=maskT[h, msl, lsl])
                                nc.vector.tensor_add(
                                    ps_s[:, hh * LC : (hh + 1) * LC],
                                    ps_s[:, hh * LC : (hh + 1) * LC],
                                    mk,
                                )
                        ps_s_q.append(ps_s)

                    s_pair(0)
                    for mt in range(N_LT):
                        if mt + 1 < N_LT:
                            s_pair(mt + 1)
                        if pending_proj and mt >= 4 and mt % 3 == 1:
                            project_group(*pending_proj.pop(0))
                        ps_s = ps_s_q.pop(0)
                        pt = work.tile([128, 2 * LC], DT, name="pt", tag="pt", bufs=4)
                        nc.scalar.activation(pt, ps_s, Exp)
                        for hh in range(2):
                            h = 2 * hp + hh
                            nc.tensor.matmul(
                                po[hh][0 : D + 1, :],
                                lhsT=v_sb[:, mt, h, :],
                                rhs=pt[:, hh * LC : (hh + 1) * LC],
                                start=(mt == 0),
                                stop=(mt == N_LT - 1),
                            )
                    last = lc == N_LC - 1 and hp == 1
                    rbs = []
                    for hh in range(2):
                        off = 64 * hh
                        # Normalize without leaving the chip: DVE
                        # approx-reciprocal of the denominator row (~18
                        # correct bits, 5x cheaper than exact), gpsimd
                        # partition_broadcast fans it out to 64 partitions,
                        # one DVE multiply writes oT. Neither the PE nor any
                        # DMA queue is in this chain, so the PE never idles
                        # long enough for a HAM re-throttle. The O block is
                        # copied out of PSUM first to release the po bank so
                        # the next head pair's A*V never stalls — except on
                        # the very last chain, where nothing follows and
                        # multiplying straight from PSUM shortens the tail.
                        # (reciprocal_approx_fast requires its input at
                        # partition offset 0 — the custom-DVE op misreads
                        # offset APs on HW — so the denominator row gets its
                        # own partition-0 staging copy)
                        dr = work.tile([1, LC], F32, name="dr", tag="dr", bufs=2)
                        nc.vector.tensor_copy(dr, po[hh][D : D + 1, :])
                        if not last:
                            dn = work.tile([64, LC], F32, name="dn", tag="dn", bufs=3)
                            nc.vector.tensor_copy(dn, po[hh][0:D, :])
                        rr = work.tile([1, LC], F32, name="rr", tag="rr", bufs=2)
                        nc.vector.reciprocal_approx_fast(rr, dr)
                        rb = work.tile([64, LC], F32, name="rb", tag="rb", bufs=2)
                        nc.gpsimd.partition_broadcast(rb, rr)
                        if last:
                            rbs.append(rb)
                        else:
                            nc.vector.tensor_mul(
                                oT[off : off + 64, hp, :], dn, rb
                            )
                    if last:
                        # tail pipelining: each projection row-tile needs only
                        # its own 128-column segment of oT, so normalize
                        # segment-by-segment (straight from PSUM) and emit
                        # that segment's projection immediately — the PE
                        # projects segment k while the DVE normalizes k+1
                        for seg in range(LC // 128):
                            csl = slice(seg * 128, (seg + 1) * 128)
                            for hh in range(2):
                                off = 64 * hh
                                nc.vector.tensor_mul(
                                    oT[off : off + 64, hp, csl],
                                    po[hh][0:D, csl],
                                    rbs[hh][:, csl],
                                )
                            project_group(oT, lc * (LC // 128) + seg, direct=(seg % 2 == 1))
                    elif hp == 1:
                        pending_proj += [
                            (oT, lt)
                            for lt in range(lc * LC // 128, (lc + 1) * LC // 128)
                        ]
            for args in pending_proj:
                project_group(*args)

    nc.compile()
    _build_cache[with_mask] = nc
    return nc


def _prepare_in_maps(x, attn_mask, qkv_w, proj_w, s, with_mask):
    qk_scale = D ** -0.5
    q_scale = qk_scale * float(s) * math.log(L)
    x = np.asarray(x, np.float32)
    qkv_w = np.asarray(qkv_w, np.float32)
    proj_w = np.asarray(proj_w, np.float32)

    in_maps = []
    for core in range(N_CORES):
        b = core // (N_CORES // B)
        h0 = (core % (N_CORES // B)) * HL
        fs = slice(h0 * D, h0 * D + F)
        wq = qkv_w[0 * DIM : 1 * DIM][fs] * q_scale  # [F, DIM]
        wk = qkv_w[1 * DIM : 2 * DIM][fs]
        wvm = qkv_w[2 * DIM : 3 * DIM][fs]
        m = {
            "xT": np.ascontiguousarray(x[b].T),
            "wqk": np.ascontiguousarray(np.concatenate([wq, wk], axis=0).T),
            "wv": np.ascontiguousarray(wvm.T),
            "wp": np.ascontiguousarray(proj_w[:, fs].T),
        }
        if with_mask:
            m["maskT"] = np.ascontiguousarray(
                np.transpose(attn_mask[b, h0 : h0 + HL], (0, 2, 1))
            ).astype(np.float32)
        in_maps.append(m)
    return in_maps


def _postprocess(results, proj_b):
    gpb = N_CORES // B
    y = np.zeros((B, L, DIM), np.float32)
    for core in range(N_CORES):
        y[core // gpb] += results[core]["y"]
    y += np.asarray(proj_b, np.float32)[None, None, :]
    return y


def run(x, attn_mask, qkv_w, proj_w, proj_b, s, **spmd_kwargs):
    with_mask = bool(np.any(attn_mask))
    nc = _build(with_mask)
    in_maps = _prepare_in_maps(x, attn_mask, qkv_w, proj_w, s, with_mask)
    res = bass_utils.run_bass_kernel_spmd(
        nc, in_maps, core_ids=list(range(N_CORES)), **spmd_kwargs
    )
    return _postprocess(res.results, proj_b), res


def kernel(x, attn_mask, qkv_w, proj_w, proj_b, s):
    y, _ = run(x, attn_mask, qkv_w, proj_w, proj_b, s)
    return y



# revision 12
# speedup vs baseline: 1.1585x; 1.1585x over previous
"""Trainium2 Bass kernel for nn_Attention (B=2, L=2048, DIM=1024, H=16, D=64).

Sharding: 8 cores, each handles one (b, 4-head-group) pair — data parallel
on B (cores 0-3 -> b=0, cores 4-7 -> b=1), tensor parallel on heads
(4 heads per core). The output projection is computed per-core over the
core's 4 heads; the host sums the 4 partials per batch and adds the bias.

All matmuls run in float32r (single-pass fp32 PE mode, ~bf16 speed at
N>=512, ~1.6e-4 matmul rel err). DRAM inputs are declared float32r
directly (bit-identical f32 storage; the PE truncates instead of rounds
the mantissa, a negligible 2^-19 difference), so every load is a plain
DMA with no staging or cast step.

Device-side layout (per core, contraction dim on SBUF partitions):
  xT   [DIM, L]     x[b]^T
  wqk  [DIM, 512]   [Wq_scaled | Wk]^T for the core's 4 heads (Wq pre-scaled
                    by qk_scale * s * log(L) so exp() needs no extra scale)
  wv   [DIM, 256]   Wv^T for the 4 heads
  wp   [256, DIM]   proj_w[:, head_slice]^T
  y    [L, DIM]     per-core partial output (pre-bias)

Schedule (PE-bubble-free to avoid HAM duty-cycle re-throttle: any PE idle
window >3.4us drops the PE clock from 2.4 to 1.2 GHz for the next ~3.4us+;
the warmup matmuls are sized so the clock is hot when wqk lands):
 - QKV phase consumes x^T chunk-by-chunk in DMA arrival order: per l-chunk
   {q ft0, k ft2, V 4 l-tiles}, then q/k for heads 2-3 (ft1/ft3) at the
   tail of the (DMA-bound) phase. Inputs ride two concurrent DMA queues
   with one dma_start per tensor chunk (per-dma dispatch costs ~1us on
   the SWDGE queue; small DMAs starve the matmul stream).
 - Attention per head pair: S^T tiles for both heads land in one
   [128,1024] PSUM pair tile (the two K=64 matmuls dual-issue on PE row
   groups h0/h64, so S costs half its naive time); one ACT exp converts
   the pair; A*V via matmul(lhsT=[V | ones], rhs=P^T) also yields softmax
   denominators in row D of the PSUM accumulator. The attention steady
   state is ACT-exp-bound (~1.13us per m-tile, ~99% ACT occupancy).
 - Normalize with no DRAM roundtrip and no PE involvement: the [O; denom]
   block leaves PSUM via one DVE copy (releasing the po bank so the next
   head pair's A*V never stalls), DVE reciprocal_approx_fast inverts the
   denominator row (input staged to partition 0 — the custom-DVE op
   misreads partition-offset APs on HW), gpsimd partition_broadcast fans
   it out, one DVE multiply writes oT. No DMA queue or PE in the chain,
   so chunk-boundary PE stalls stay well under the HAM window.
 - The projection is software-pipelined one l-chunk behind, interleaved
   into the next chunk's matmul stream; oT lives in per-chunk pool tiles
   so chunk i's projection has no false dependency on chunk i+1's
   normalize. The very last normalize chain skips the PSUM-release copy
   and multiplies straight from PSUM to shorten the tail.
"""

import math
import sys

sys.path.insert(0, "/opt/trn_rl_repo")

import numpy as np

import concourse.bass as bass
import concourse.tile as tile
from concourse import bacc, bass_utils, mybir

B, L, DIM, H, D = 2, 2048, 1024, 16, 64
N_CORES = 8
HL = 4  # heads per core
F = HL * D  # 256: per-core head feature width
LC, LT, CT = 512, 128, 128  # l-chunk, l/m-tile, contraction tile
N_LC, N_LT, N_CT = L // LC, L // LT, DIM // CT

DT = mybir.dt.float32r
BF16 = mybir.dt.bfloat16
F32 = mybir.dt.float32

_build_cache = {}


def _build(with_mask: bool):
    if with_mask in _build_cache:
        return _build_cache[with_mask]

    nc = bacc.Bacc("TRN2", target_bir_lowering=False, debug=False, num_devices=N_CORES)
    # inputs declared float32r: bit-identical f32 storage (np maps both to
    # float32), so plain DMAs load them with no cast step. The PE's f32r
    # mode truncates instead of rounds the mantissa — a 2^-19 relative
    # difference, far below the matmul's own ~1.6e-4 error.
    xT = nc.dram_tensor("xT", [DIM, L], DT, kind="ExternalInput").ap()
    wqk = nc.dram_tensor("wqk", [DIM, 2 * F], DT, kind="ExternalInput").ap()
    wv = nc.dram_tensor("wv", [DIM, F], DT, kind="ExternalInput").ap()
    wp = nc.dram_tensor("wp", [F, DIM], DT, kind="ExternalInput").ap()
    if with_mask:
        maskT = nc.dram_tensor("maskT", [HL, L, L], F32, kind="ExternalInput").ap()
    y = nc.dram_tensor("y", [L, DIM], F32, kind="ExternalOutput").ap()

    Exp = mybir.ActivationFunctionType.Exp

    with tile.TileContext(nc) as tc:
        with (
            tc.tile_pool(name="consts", bufs=1) as consts,
            tc.tile_pool(name="work", bufs=3) as work,
            tc.tile_pool(name="otp", bufs=2) as otp,
            tc.tile_pool(name="ps_mm", bufs=3, space="PSUM") as ps_mm,
            tc.tile_pool(name="ps_acc", bufs=2, space="PSUM") as ps_acc,
        ):
            # ---- PE warmup: dummy matmuls during input DMA so the HAM
            # clock-gate reaches 2.4 GHz before the real work starts ----
            warm = consts.tile([128, 512], mybir.dt.bfloat16)
            nc.vector.memset(warm, 0.0)
            ps_w = ps_acc.tile([128, 512], F32, name="ps_w", tag="acc")
            n_warm = 62  # sized to end as wqk lands (~26us): the clock is
            # warm when real work starts, with no wasted PE time after
            for i in range(n_warm):
                nc.tensor.matmul(
                    ps_w, lhsT=warm[:, 0:128], rhs=warm,
                    start=(i == 0), stop=(i == n_warm - 1),
                )

            # ---- load inputs. All tensors are f32r in DRAM (bit-identical
            # f32), so every load is a plain DMA — no staging, no DVE cast.
            # Split across the two queues so they pull concurrently; one
            # dma_start per tensor chunk (per-dma dispatch costs ~1us on
            # the SWDGE queue). Order matters: HBM bandwidth is the
            # limiter, so urgent first. ----
            xT_sb = consts.tile([128, N_CT, L], DT)
            wv_sb = consts.tile([128, N_CT, F], DT)
            wp_sb = consts.tile([128, 2, DIM], DT)
            wqk_sb = consts.tile([128, N_CT, 2 * F], DT)
            # gpsimd: wqk first (gates all q/k groups), wv (needed by the
            # first V tiles), then x^T lc2-3, wp last (needed mid-attention)
            for h in range(2):
                hc = N_CT // 2
                src2 = bass.AP(
                    tensor=wqk.tensor,
                    offset=h * hc * 128 * 2 * F,
                    ap=[[2 * F, 128], [128 * 2 * F, hc], [1, 2 * F]],
                )
                nc.gpsimd.dma_start(out=wqk_sb[:, h * hc : (h + 1) * hc, :], in_=src2)
            srcv = bass.AP(
                tensor=wv.tensor,
                offset=0,
                ap=[[F, 128], [128 * F, N_CT], [1, F]],
            )
            nc.gpsimd.dma_start(out=wv_sb, in_=srcv)
            # sync: x^T lc0 and lc1
            for lc in range(2):
                srcx = bass.AP(
                    tensor=xT.tensor,
                    offset=lc * LC,
                    ap=[[L, 128], [128 * L, N_CT], [1, LC]],
                )
                nc.sync.dma_start(
                    out=xT_sb[:, :, lc * LC : (lc + 1) * LC], in_=srcx
                )
            for lc in range(2, N_LC):
                srcx = bass.AP(
                    tensor=xT.tensor,
                    offset=lc * LC,
                    ap=[[L, 128], [128 * L, N_CT], [1, LC]],
                )
                nc.gpsimd.dma_start(out=xT_sb[:, :, lc * LC : (lc + 1) * LC], in_=srcx)
            nc.gpsimd.dma_start(out=wp_sb, in_=wp[0:256, :].rearrange("(t p) o -> p t o", t=2))

            # ---- stage A: Q^T/K^T [f, l] (f = [q 4 heads | k 4 heads] * 64)
            # and V, interleaved per l-chunk in DMA arrival order ----
            qkT_sb = consts.tile([128, 4, L], DT)
            v_sb = consts.tile([128, N_LT, HL, D + 1], DT)
            ones_f32 = consts.tile([128, 64], F32)
            nc.vector.memset(ones_f32, 1.0)
            nc.vector.tensor_copy(
                v_sb[:, :, :, D : D + 1],
                ones_f32.rearrange("p (a b c) -> p a b c", a=N_LT, b=HL),
            )

            def qk_group(ft, lc):
                ps = ps_mm.tile([128, LC], F32, name="ps_qk", tag="mm")
                for c in range(N_CT):
                    nc.tensor.matmul(
                        ps,
                        lhsT=wqk_sb[:, c, ft * 128 : (ft + 1) * 128],
                        rhs=xT_sb[:, c, lc * LC : (lc + 1) * LC],
                        start=(c == 0),
                        stop=(c == N_CT - 1),
                    )
                nc.vector.tensor_copy(qkT_sb[:, ft, lc * LC : (lc + 1) * LC], ps)

            def v_tile(lt):
                ps = ps_mm.tile([128, F], F32, name="ps_v", tag="mm")
                for c in range(N_CT):
                    nc.tensor.matmul(
                        ps,
                        lhsT=xT_sb[:, c, lt * 128 : (lt + 1) * 128],
                        rhs=wv_sb[:, c, :],
                        start=(c == 0),
                        stop=(c == N_CT - 1),
                    )
                nc.vector.tensor_copy(
                    v_sb[:, lt, :, 0:D], ps.rearrange("p (h d) -> p h d", h=HL)
                )

            for lc in range(N_LC):
                qk_group(0, lc)
                qk_group(2, lc)
                for lt in range(lc * (LC // 128), (lc + 1) * (LC // 128)):
                    v_tile(lt)

            # q/k for heads 2-3 at the tail of the (DMA-bound) QKV phase:
            # they need no new data, and keeping them out of the attention
            # phase keeps the per-m-tile PE load at or below the ACT exp
            # rate there (the attention steady state is ACT-paced)
            for ft in (1, 3):
                for lc in range(N_LC):
                    qk_group(ft, lc)

            # ---- stage B + C fused: attention, with the projection software-
            # pipelined one l-chunk behind so the PE never stalls on the
            # normalize chain (stalls re-throttle the PE clock via HAM) ----
            def project_group(oT, lt, direct=False):
                # one [128, 1024] output row-tile per group; each matmul
                # output stays within a single 2KB PSUM bank (N=512 max)
                ltl = lt % (LC // 128)
                ps = ps_mm.tile([128, DIM], F32, name="ps_y", tag="mm")
                for oc in range(2):
                    osl = slice(oc * 512, (oc + 1) * 512)
                    for ft in range(2):
                        nc.tensor.matmul(
                            ps[:, osl],
                            lhsT=oT[:, ft, ltl * 128 : (ltl + 1) * 128],
                            rhs=wp_sb[:, ft, osl],
                            start=(ft == 0),
                            stop=(ft == 1),
                        )
                yb = work.tile([128, DIM], F32, name="yb", tag="yb", bufs=2)
                if direct:
                    # final groups: the ACT engine is idle by now, so stage
                    # through it instead of queueing behind the DVE copies
                    nc.scalar.copy(yb, ps)
                else:
                    nc.vector.tensor_copy(yb, ps)
                nc.sync.dma_start(out=y[lt * 128 : (lt + 1) * 128, :], in_=yb)

            # (oT, lt, oc) projection groups for the previous l-chunk, emitted
            # interleaved into the current chunk's matmul stream
            pending_proj = []
            for lc in range(N_LC):
                lsl = slice(lc * LC, (lc + 1) * LC)
                oT = otp.tile([128, 2, LC], DT, name="oT", tag="ot")
                for hp in range(2):  # head pairs (2*hp, 2*hp+1)
                    po = [
                        ps_acc.tile([128, LC], F32, name="po", tag="acc")
                        for _ in range(2)
                    ]
                    ps_s_q = []

                    def s_pair(mt):
                        msl = slice(mt * 128, (mt + 1) * 128)
                        ps_s = ps_mm.tile([128, 2 * LC], F32, name="ps_s", tag="mm")
                        for hh in range(2):
                            off = 64 * hh
                            nc.tensor.matmul(
                                ps_s[:, hh * LC : (hh + 1) * LC],
                                lhsT=qkT_sb[off : off + 64, 2 + hp, msl],
                                rhs=qkT_sb[off : off + 64, hp, lsl],
                                start=True,
                                stop=True,
                            )
                        if with_mask:
                            for hh in range(2):
                                h = 2 * hp + hh
                                mk = work.tile([128, LC], F32, name="mk", tag="mk", bufs=4)
                                nc.sync.dma_start(out=mk, in_=maskT[h, msl, lsl])
                                nc.vector.tensor_add(
                                    ps_s[:, hh * LC : (hh + 1) * LC],
                                    ps_s[:, hh * LC : (hh + 1) * LC],
                                    mk,
                                )
                        ps_s_q.append(ps_s)

                    s_pair(0)
                    for mt in range(N_LT):
                        if mt + 1 < N_LT:
                            s_pair(mt + 1)
                        if pending_proj and mt >= 4 and mt % 3 == 1:
                            project_group(*pending_proj.pop(0))
                        ps_s = ps_s_q.pop(0)
                        pt = work.tile([128, 2 * LC], DT, name="pt", tag="pt", bufs=4)
                        nc.scalar.activation(pt, ps_s, Exp)
                        for hh in range(2):
                            h = 2 * hp + hh
                            nc.tensor.matmul(
                                po[hh][0 : D + 1, :],
                                lhsT=v_sb[:, mt, h, :],
                                rhs=pt[:, hh * LC : (hh + 1) * LC],
                                start=(mt == 0),
                                stop=(mt == N_LT - 1),
                            )
                    last = lc == N_LC - 1 and hp == 1
                    rbs = []
                    for hh in range(2):
                        off = 64 * hh
                        # Normalize without leaving the chip: DVE
                        # approx-reciprocal of the denominator row (~18
                        # correct bits, 5x cheaper than exact), gpsimd
                        # partition_broadcast fans it out to 64 partitions,
                        # one DVE multiply writes oT. Neither the PE nor any
                        # DMA queue is in this chain, so the PE never idles
                        # long enough for a HAM re-throttle. The O block is
                        # copied out of PSUM first to release the po bank so
                        # the next head pair's A*V never stalls — except on
                        # the very last chain, where nothing follows and
                        # multiplying straight from PSUM shortens the tail.
                        # (reciprocal_approx_fast requires its input at
                        # partition offset 0 — the custom-DVE op misreads
                        # offset APs on HW — so the denominator row gets its
                        # own partition-0 staging copy)
                        dr = work.tile([1, LC], F32, name="dr", tag="dr", bufs=2)
                        nc.vector.tensor_copy(dr, po[hh][D : D + 1, :])
                        if not last:
                            dn = work.tile([64, LC], F32, name="dn", tag="dn", bufs=3)
                            nc.vector.tensor_copy(dn, po[hh][0:D, :])
                        rr = work.tile([1, LC], F32, name="rr", tag="rr", bufs=2)
                        nc.vector.reciprocal_approx_fast(rr, dr)
                        rb = work.tile([64, LC], F32, name="rb", tag="rb", bufs=2)
                        nc.gpsimd.partition_broadcast(rb, rr)
                        if last:
                            rbs.append(rb)
                        else:
                            nc.vector.tensor_mul(
                                oT[off : off + 64, hp, :], dn, rb
                            )
                    if last:
                        # tail pipelining: each projection row-tile needs only
                        # its own 128-column segment of oT, so normalize
                        # segment-by-segment (straight from PSUM) and emit
                        # that segment's projection immediately — the PE
                        # projects segment k while the DVE normalizes k+1
                        for seg in range(LC // 128):
                            csl = slice(seg * 128, (seg + 1) * 128)
                            for hh in range(2):
                                off = 64 * hh
                                nc.vector.tensor_mul(
                                    oT[off : off + 64, hp, csl],
                                    po[hh][0:D, csl],
                                    rbs[hh][:, csl],
                                )
                            project_group(oT, lc * (LC // 128) + seg, direct=(seg % 2 == 1))
                    elif hp == 1:
                        pending_proj += [
                            (oT, lt)
                            for lt in range(lc * LC // 128, (lc + 1) * LC // 128)
                        ]
            for args in pending_proj:
                project_group(*args)

    nc.compile()
    _build_cache[with_mask] = nc
    return nc


def _prepare_in_maps(x, attn_mask, qkv_w, proj_w, s, with_mask):
    qk_scale = D ** -0.5
    q_scale = qk_scale * float(s) * math.log(L)
    x = np.asarray(x, np.float32)
    qkv_w = np.asarray(qkv_w, np.float32)
    proj_w = np.asarray(proj_w, np.float32)

    in_maps = []
    for core in range(N_CORES):
        b = core // (N_CORES // B)
        h0 = (core % (N_CORES // B)) * HL
        fs = slice(h0 * D, h0 * D + F)
        wq = qkv_w[0 * DIM : 1 * DIM][fs] * q_scale  # [F, DIM]
        wk = qkv_w[1 * DIM : 2 * DIM][fs]
        wvm = qkv_w[2 * DIM : 3 * DIM][fs]
        m = {
            "xT": np.ascontiguousarray(x[b].T),
            "wqk": np.ascontiguousarray(np.concatenate([wq, wk], axis=0).T),
            "wv": np.ascontiguousarray(wvm.T),
            "wp": np.ascontiguousarray(proj_w[:, fs].T),
        }
        if with_mask:
            m["maskT"] = np.ascontiguousarray(
                np.transpose(attn_mask[b, h0 : h0 + HL], (0, 2, 1))
            ).astype(np.float32)
        in_maps.append(m)
    return in_maps


def _postprocess(results, proj_b):
    gpb = N_CORES // B
    y = np.zeros((B, L, DIM), np.float32)
    for core in range(N_CORES):
        y[core // gpb] += results[core]["y"]
    y += np.asarray(proj_b, np.float32)[None, None, :]
    return y


def run(x, attn_mask, qkv_w, proj_w, proj_b, s, **spmd_kwargs):
    with_mask = bool(np.any(attn_mask))
    nc = _build(with_mask)
    in_maps = _prepare_in_maps(x, attn_mask, qkv_w, proj_w, s, with_mask)
    res = bass_utils.run_bass_kernel_spmd(
        nc, in_maps, core_ids=list(range(N_CORES)), **spmd_kwargs
    )
    return _postprocess(res.results, proj_b), res


def kernel(x, attn_mask, qkv_w, proj_w, proj_b, s):
    y, _ = run(x, attn_mask, qkv_w, proj_w, proj_b, s)
    return y

